# revision 48
# baseline (speedup 1.0000x reference)
"""Trainium2 Bass kernel for nn_Encoder (S=4096, D=512, H=8, E=64).

Sharding: sequence-parallel over 8 cores with distributed K/V: each core
projects Q/K/V only for its OWN 512 rows, then four pipelined AllGather
slices (~0.26 MB/rank each) assemble the full K^T and V' on every core
while attention is already consuming the earlier slices. Attention, the
output projection, the global LayerNorms (two 8-byte AllGathers for the
joint [S, D] statistics) and the MLP run on the own-row shard. The host
concatenates the per-core row shards.

Scheduling structure:
  - a dummy 4-byte AllGather is issued first so the one-time collective
    rendezvous barrier + first-collective setup cost runs during the
    startup loads instead of before the K/V gather
  - slice j carries K^T for head-pair j plus V' for key-chunk column j;
    the attention pass loop iterates key chunks column-outer so slice j
    is consumed j-th
  - AV matmuls run one chunk behind the logits/exp stream so the
    in-order PE queue never stalls on the current chunk's exp
  - Kp/Vp (the K/V side outputs) fill the AllGather latency window
  - when ln_g == 1 and ln_b == 0 (the reference initializer), LN1's
    affine commutes into the MLP first matmul: z^T @ W1 is computed
    during the LN1-stats collective and h1 = relu(rstd*(z^T@W1) +
    (b1 - mu*rstd*colsum(W1))) needs only one activation per f-tile.
"""

import os

os.environ.setdefault("JAX_PLATFORMS", "axon")

import numpy as np
import ml_dtypes

import concourse.bass as bass
import concourse.tile as tile
from concourse import mybir
from concourse.bass_utils import run_bass_kernel_spmd
from concourse.masks import make_identity

dt = mybir.dt
AF = mybir.ActivationFunctionType
ALU = mybir.AluOpType
AX = mybir.AxisListType

N_CORES = 8
S, D, H, E = 4096, 512, 8, 64
F = 4 * D          # 2048
R = S // N_CORES   # 512 rows per core
EPS = 1e-5
SCALE = 1.0 / float(np.sqrt(E))
INV_SD = 1.0 / float(S * D)

SL_KT = 2 * 512              # K^T slice dump per partition [2mc, t]
SL_VP = 2 * 8 * 65           # V' slice dump per partition [2tc, h, e']
SL_X = SL_KT + SL_VP         # per-partition elems of one AG slice
SL_ELEMS = 128 * SL_X        # one AG slice, per rank

BF16 = ml_dtypes.bfloat16


def split_waits(nc):
    """Walrus codegen allows only one sync-wait per HW instruction. Move
    extra waits onto single-wait NoOps inserted before, same engine queue."""
    import bass_rust

    n = 0
    for bb in nc.m.functions[0].blocks:
        new_list = []
        changed = False
        for ins in bb.instructions:
            si = ins.sync_info
            if si is not None and si.on_wait is not None and len(si.on_wait) > 1:
                waits = list(si.on_wait)
                for w in waits[:-1]:
                    nop = bass_rust.InstNoOp(name=f"I-xwait-{n}")
                    n += 1
                    nop.engine = ins.engine
                    nop.sync_info = bass_rust.SyncInfo(on_wait=[w], on_update=[])
                    nc.register_instruction(nop)
                    new_list.append(nop)
                si.on_wait = waits[-1:]
                ins.sync_info = si
                changed = True
            new_list.append(ins)
        if changed:
            bb.instructions = new_list
    return nc


def build_nc(fast_ln=True):
    import contextlib

    nc = bass.Bass("TRN2", debug=False, num_devices=N_CORES)
    f32, f32r, bf16 = dt.float32, dt.float32r, dt.bfloat16
    f8 = dt.float8e4

    # ---- I/O (all host-prepped layouts) -------------------------------
    xro_d = nc.dram_tensor("xro", [128, 4, D], f32, kind="ExternalInput").ap()
    xrT_d = nc.dram_tensor("xrT", [128, 4, R], bf16, kind="ExternalInput").ap()
    wq_d = nc.dram_tensor("wq", [128, 4, D], bf16, kind="ExternalInput").ap()
    wk_d = nc.dram_tensor("wk", [128, 4, D], bf16, kind="ExternalInput").ap()
    wv_d = nc.dram_tensor("wv", [128, 4, D], bf16, kind="ExternalInput").ap()
    wos_d = nc.dram_tensor("wo_s", [64, H, D], bf16, kind="ExternalInput").ap()
    wop_d = nc.dram_tensor("wo_p", [128, 4, D], bf16, kind="ExternalInput").ap()
    w1_d = nc.dram_tensor("w1", [128, 4, F], bf16, kind="ExternalInput").ap()
    w2_d = nc.dram_tensor("w2", [128, 4, 4, D], bf16, kind="ExternalInput").ap()
    cs1_d = nc.dram_tensor("colsum_w1", [128, 16], f32, kind="ExternalInput").ap()
    bqs_d = nc.dram_tensor("bqs2", [128, 4], f32, kind="ExternalInput").ap()
    bks2_d = nc.dram_tensor("bks2", [128, 4], f32, kind="ExternalInput").ap()
    bvs2_d = nc.dram_tensor("bvs2", [128, 4], f32, kind="ExternalInput").ap()
    bvbc_d = nc.dram_tensor("bv_bc", [128, D], f32, kind="ExternalInput").ap()
    b1s_d = nc.dram_tensor("b1s", [128, 16], f32, kind="ExternalInput").ap()
    bor_d = nc.dram_tensor("bo_r", [1, D], bf16, kind="ExternalInput").ap()
    b2r_d = nc.dram_tensor("b2_r", [1, D], bf16, kind="ExternalInput").ap()
    if not fast_ln:
        gnat_d = nc.dram_tensor("g_nat", [128, 4, D], bf16, kind="ExternalInput").ap()
        bnat_d = nc.dram_tensor("b_nat", [128, 4, D], bf16, kind="ExternalInput").ap()
        gT_d = nc.dram_tensor("gT", [128, 4, R], bf16, kind="ExternalInput").ap()
        bT_d = nc.dram_tensor("bT", [128, 4, R], bf16, kind="ExternalInput").ap()

    fin_d = nc.dram_tensor("final_rows", [R, D], f32, kind="ExternalOutput").ap()
    kp_d = nc.dram_tensor("Kp_rows", [R, D], f32, kind="ExternalOutput").ap()
    vp_d = nc.dram_tensor("Vp_rows", [R, D], f32, kind="ExternalOutput").ap()

    # row index q = qc*128 + p everywhere
    fin_v = fin_d.rearrange("(c p) d -> p c d", p=128)
    kp_v = kp_d.rearrange("(c p) d -> p c d", p=128)
    vp_v = vp_d.rearrange("(c p) d -> p c d", p=128)

    with tile.TileContext(nc) as tc, contextlib.ExitStack() as ctx, \
            nc.allow_low_precision(reason="bf16 matmul operands, fp32 accumulate"):
        ep = ctx.enter_context

        # ---- pools ----------------------------------------------------
        single = ep(tc.tile_pool(name="single", bufs=1))
        wpool = ep(tc.tile_pool(name="wpool", bufs=1))
        kt_p = ep(tc.tile_pool(name="ktp", bufs=2))      # K^T pair per pass
        vp_p = ep(tc.tile_pool(name="vpp", bufs=4))      # V' per tc slice
        pexp_p = ep(tc.tile_pool(name="pexp", bufs=3))
        evac = ep(tc.tile_pool(name="evac", bufs=3))
        otr_p = ep(tc.tile_pool(name="otr", bufs=2))
        wk_p = ep(tc.tile_pool(name="wk", bufs=2))
        # psum: tag "mm" 2x3banks + tag "po" 2x1bank = 8 banks
        ps_mm = ep(tc.tile_pool(name="ps_mm", bufs=2, space="PSUM"))
        ps_po = ep(tc.tile_pool(name="ps_po", bufs=2, space="PSUM"))
        dram = ep(tc.tile_pool(name="dram", bufs=1, space="DRAM"))

        # AllGather slice buffers: slice j = K^T pairs 2j,2j+1 + V' cols 2j,2j+1
        ag_in, ag_out, agi, ago_kt, ago_vp = [], [], [], [], []
        for j in range(2):
            ai = dram.tile([SL_ELEMS], f8, name=f"ag_in{j}")
            ao = dram.tile([N_CORES, SL_ELEMS], f8, addr_space="Shared",
                           name=f"ag_out{j}")
            ag_in.append(ai)
            ag_out.append(ao)
            agi.append(ai[:].rearrange("(p x) -> p x", p=128))
            av = ao[:].rearrange("r (p x) -> p r x", p=128)
            ago_kt.append(av[:, :, 0:SL_KT])
            ago_vp.append(av[:, :, SL_KT:SL_X])

        # ---- constants ------------------------------------------------
        ident = single.tile([128, 128], f32)
        make_identity(nc, ident[:])
        ones1 = single.tile([1, 128], f32)
        nc.vector.memset(ones1[:], 1.0)
        ones_row = single.tile([1, 128], bf16)
        nc.vector.tensor_copy(ones_row[:], ones1[:])
        ones_row_r = single.tile([1, 128], f32r)
        nc.vector.tensor_copy(ones_row_r[:], ones1[:])
        ones8 = single.tile([128, 8], bf16)
        nc.vector.memset(ones8[:], 1.0)
        onesP = single.tile([128, 1], f32)
        nc.vector.memset(onesP[:], 1.0)
        eps_t = single.tile([1, 1], f32)
        nc.vector.memset(eps_t[:], EPS)
        shift_t = single.tile([128, 1], f32)
        nc.vector.memset(shift_t[:], -3.0)

        # ---- loads on the AG critical path first ----------------------
        xrT = single.tile([128, 4, R], bf16)      # x^T own rows (host prepped)
        nc.sync.dma_start(xrT[:], xrT_d)
        w_k = wpool.tile([128, 4, D], bf16)
        nc.sync.dma_start(w_k[:], wk_d)
        w_v = wpool.tile([128, 4, D], bf16)
        nc.sync.dma_start(w_v[:], wv_d)
        bks2 = single.tile([128, 4], f32)
        nc.sync.dma_start(bks2[:], bks2_d)
        bvs2 = single.tile([128, 4], f32)
        nc.sync.dma_start(bvs2[:], bvs2_d)
        bv_bc = single.tile([128, D], f32)
        nc.sync.dma_start(bv_bc[:], bvbc_d)

        # ---- phase 1: own-row K/V projections, sliced + gathered ------
        KTo = single.tile([128, 4, R], bf16)      # kept for Kp
        VPo = single.tile([128, 4, H, E + 1], bf16)
        for col in range(4):
            pq = ps_mm.tile([128, R], f32, tag="mm")
            for dc in range(4):
                nc.tensor.matmul(
                    pq[:],
                    lhsT=w_k[:, dc, col * 128:(col + 1) * 128],
                    rhs=xrT[:, dc, :],
                    start=(dc == 0), stop=(dc == 3),
                )
            nc.vector.tensor_scalar_add(KTo[:, col, :], pq[:], bks2[:, col:col + 1])
            pv = ps_mm.tile([128, D], f32, tag="mm")
            for dc in range(4):
                nc.tensor.matmul(
                    pv[:],
                    lhsT=xrT[:, dc, col * 128:(col + 1) * 128],
                    rhs=w_v[:, dc, :],
                    start=(dc == 0), stop=(dc == 3),
                )
            nc.vector.tensor_tensor(
                VPo[:, col, :, 0:E],
                pv[:].rearrange("p (h e) -> p h e", e=E),
                bv_bc[:].rearrange("p (h e) -> p h e", e=E),
                ALU.add,
            )
            nc.vector.tensor_copy(VPo[:, col, :, E], ones8[:])
            j, c2 = col // 2, col % 2
            nc.gpsimd.dma_start(
                agi[j][:, c2 * 512:(c2 + 1) * 512], KTo[:, col, :]
            )
            nc.gpsimd.dma_start(
                agi[j][:, SL_KT + c2 * 520:SL_KT + (c2 + 1) * 520].rearrange(
                    "p (h e) -> p h e", e=E + 1),
                VPo[:, col, :, :],
            )
            if c2 == 1:
                nc.gpsimd.collective_compute(
                    "AllGather", ALU.bypass,
                    replica_groups=[list(range(N_CORES))],
                    ins=[ag_in[j][:]], outs=[ag_out[j][:]],
                )

        # ---- AG window: remaining loads, Q^T, V^T packed, Kp, Vp ------
        w_q = wpool.tile([128, 4, D], bf16)
        nc.sync.dma_start(w_q[:], wq_d)
        bqs2 = single.tile([128, 4], f32)
        nc.sync.dma_start(bqs2[:], bqs_d)
        Wo_p = wpool.tile([128, 4, D], bf16)
        nc.sync.dma_start(Wo_p[:], wop_d)
        bo_r = single.tile([1, D], bf16)
        nc.sync.dma_start(bo_r[:], bor_d)
        b2_r = single.tile([1, D], bf16)
        nc.sync.dma_start(b2_r[:], b2r_d)
        b1s = single.tile([128, 16], f32)
        nc.sync.dma_start(b1s[:], b1s_d)
        cs1 = single.tile([128, 16], f32)
        nc.sync.dma_start(cs1[:], cs1_d)

        # QT[:, h, :] holds Q_h^T at rows (h%2)*64..+64, zeros elsewhere, so
        # a pair-packed K^T block serves as shared lhsT for both heads.
        QT = single.tile([128, H, R], f8)
        nc.vector.memset(QT[:], 0.0)
        for c in range(4):
            pq = ps_mm.tile([128, R], f32, tag="mm")
            for dc in range(4):
                nc.tensor.matmul(
                    pq[0:64, :],
                    lhsT=w_q[:, dc, c * 128:c * 128 + 64],
                    rhs=xrT[:, dc, :],
                    start=(dc == 0), stop=(dc == 3),
                )
            for dc in range(4):
                nc.tensor.matmul(
                    pq[64:128, :],
                    lhsT=w_q[:, dc, c * 128 + 64:c * 128 + 128],
                    rhs=xrT[:, dc, :],
                    start=(dc == 0), stop=(dc == 3),
                )
            nc.vector.tensor_scalar_add(
                QT[0:64, 2 * c, :], pq[0:64, :], bqs2[0:64, c:c + 1]
            )
            nc.vector.tensor_scalar_add(
                QT[64:128, 2 * c + 1, :], pq[64:128, :], bqs2[64:128, c:c + 1]
            )

        def own_proj_packed(dst, w_t, bias2_t):
            """dst[128, mc, R] = pair-packed (x_rows @ W)^T + b, p=(h%2)*64+e."""
            for mc in range(4):
                pq = ps_mm.tile([128, R], f32, tag="mm")
                for dc in range(4):
                    nc.tensor.matmul(
                        pq[:],
                        lhsT=w_t[:, dc, mc * 128:(mc + 1) * 128],
                        rhs=xrT[:, dc, :],
                        start=(dc == 0), stop=(dc == 3),
                    )
                nc.vector.tensor_scalar_add(
                    dst[:, mc, :], pq[:], bias2_t[:, mc:mc + 1]
                )

        VTo = single.tile([128, 4, R], bf16)
        own_proj_packed(VTo, w_v, bvs2)

        def wo_project_packed(src_T, out_view):
            """out_view rows = concat_h(src) @ Wo + bo (src packed [128,4,R])."""
            for qc in range(4):
                po = ps_mm.tile([128, D], f32, tag="mm")
                for mc in range(4):
                    nc.tensor.matmul(
                        po[:],
                        lhsT=src_T[:, mc, qc * 128:(qc + 1) * 128],
                        rhs=Wo_p[:, mc, :],
                        start=(mc == 0), stop=False,
                    )
                nc.tensor.matmul(
                    po[:], lhsT=ones_row[:], rhs=bo_r[:], start=False, stop=True
                )
                ot = evac.tile([128, D], f32, tag="evac")
                nc.vector.tensor_copy(ot[:], po[:])
                nc.sync.dma_start(out_view[:, qc, :], ot[:])



        # ---- phase 2: attention (4 passes x 2 heads) ------------------
        OT = single.tile([64, H, R], bf16)  # normalized outH^T
        W1_s = wpool.tile([128, 4, F], bf16)
        W2_s = wpool.tile([128, 4, 4, D], bf16)
        xro = single.tile([128, 4, D], f32)       # x own rows; becomes z

        def kt_src(p):
            return ago_kt[p // 2][:, :, (p % 2) * 512:(p % 2 + 1) * 512]

        kt_s = [None] * 4
        kt_s[0] = kt_p.tile([128, N_CORES, R], f8, tag="kt", name="kt0")
        nc.sync.dma_start(kt_s[0][:], kt_src(0))

        # V' readbacks, ordered by producing slice so DMA-lane semaphore
        # thresholds line up with slice availability
        vp_s = []
        for tcol in range(4):
            t = vp_p.tile([128, N_CORES, H, E + 1], f8, tag="vps",
                          name=f"vps{tcol}")
            nc.sync.dma_start(
                t[:].rearrange("p r h e -> p r (h e)"),
                ago_vp[tcol // 2][:, :, (tcol % 2) * 520:(tcol % 2 + 1) * 520],
            )
            vp_s.append(t)

        def normalize(po_t, h, otr):
            """Finish softmax for one head: divide rows 0..63 of the AV
            accumulator by the ones-column denominator (row 64). The otr
            copy happens at pass end (frees the PSUM slot); this tail is
            deferred into the next pass's chunk stream."""
            rden = otr_p.tile([1, R], f32r, tag="rden")
            nc.vector.reciprocal(rden[:], otr[E:E + 1, :])
            pb = ps_mm.tile([E, R], f32, tag="mm")
            nc.tensor.matmul(
                pb[:], lhsT=ones_row_r[:, 0:E], rhs=rden[:],
                start=True, stop=True,
            )
            nc.vector.tensor_tensor(OT[:, h, :], otr[0:E, :], pb[:], ALU.mult)

        norm_pend = []
        for pass_ in range(4):
            h0, h1 = 2 * pass_, 2 * pass_ + 1
            kt_t = kt_s[pass_]
            if pass_ + 1 < 4:
                kt_s[pass_ + 1] = kt_p.tile(
                    [128, N_CORES, R], f8, tag="kt", name=f"kt{pass_ + 1}")
                nc.sync.dma_start(kt_s[pass_ + 1][:], kt_src(pass_ + 1))
            if pass_ == 1:
                # stream the MLP weights and x own rows during attention
                nc.sync.dma_start(W1_s[:], w1_d)
                nc.sync.dma_start(W2_s[:], w2_d)
                nc.sync.dma_start(xro[:], xro_d)
            po_a = ps_po.tile([E + 1, R], f32, tag="po")
            po_b = ps_po.tile([E + 1, R], f32, tag="po")
            # 64 (chunk, head) units per pass, grouped 3 per exp op to
            # amortize ACT's ~352-cycle per-instruction overhead. AV runs one
            # GROUP behind the logits/exp stream so the in-order PE queue
            # never stalls on the current group's exp.
            pend = None        # (pexp, [(unit info)...]) of the previous group
            pl = None
            group = []         # units in the currently-filling pl tile
            for u in range(64):
                ci, slot = u // 2, u % 2
                tc, r = ci // 8, ci % 8   # slice-col outer: slice tc first
                if u in (16, 24) and norm_pend:
                    # previous pass's normalize, deferred so its reciprocal
                    # completes before the PE queue reaches the broadcast
                    po_t, h, otr = norm_pend.pop(0)
                    normalize(po_t, h, otr)
                if pl is None:
                    pl = ps_mm.tile([128, 3, R], f32, tag="mm")
                    group = []
                nc.tensor.matmul(
                    pl[:, len(group), :],
                    lhsT=kt_t[:, r, tc * 128:(tc + 1) * 128],
                    rhs=QT[:, h0 if slot == 0 else h1, :],
                    start=True, stop=True,
                )
                group.append((tc, r, ci, slot))
                if len(group) == 3 or u == 63:
                    n = len(group)
                    pexp = pexp_p.tile([128, 3, R], f8, tag="pexp")
                    # -3.0 shift: keeps exp within fp8-e4m3 range (no inf
                    # encoding; overflow would NaN). Softmax is shift-
                    # invariant; the denominator comes through the V'
                    # ones-column so it scales identically.
                    nc.scalar.activation(
                        pexp[:, 0:n, :], pl[:, 0:n, :], AF.Exp,
                        scale=SCALE, bias=shift_t[:],
                    )
                    if pend is not None:
                        ppexp, pgroup = pend
                        for k, (ptc, pr_, pci, pslot) in enumerate(pgroup):
                            nc.tensor.matmul(
                                po_a[:] if pslot == 0 else po_b[:],
                                lhsT=vp_s[ptc][:, pr_, h0 if pslot == 0 else h1, :],
                                rhs=ppexp[:, k, :],
                                start=(pci == 0), stop=(pci == 31),
                            )
                    pend = (pexp, group)
                    pl = None
            ppexp, pgroup = pend
            for k, (ptc, pr_, pci, pslot) in enumerate(pgroup):
                nc.tensor.matmul(
                    po_a[:] if pslot == 0 else po_b[:],
                    lhsT=vp_s[ptc][:, pr_, h0 if pslot == 0 else h1, :],
                    rhs=ppexp[:, k, :],
                    start=(pci == 0), stop=(pci == 31),
                )
            # evacuate the AV accumulators now (frees the PSUM slots for the
            # next pass); the reciprocal+broadcast tail is deferred
            for po_t, h in ((po_a, h0), (po_b, h1)):
                otr = otr_p.tile([E + 1, R], f32, tag="otr")
                nc.vector.tensor_copy(otr[:], po_t[:])
                norm_pend.append((po_t, h, otr))
        # last pass: issue both reciprocals up front (DVE) so the PE can fill
        # the latency with out-proj matmuls for the already-normalized heads
        last_norm = []
        for po_t, h, otr in norm_pend:
            rden = otr_p.tile([1, R], f32r, tag="rden")
            nc.vector.reciprocal(rden[:], otr[E:E + 1, :])
            last_norm.append((h, otr, rden))
        norm_pend = []

        def finish_norm(h, otr, rden):
            pb = ps_mm.tile([E, R], f32, tag="mm")
            nc.tensor.matmul(
                pb[:], lhsT=ones_row_r[:, 0:E], rhs=rden[:],
                start=True, stop=True,
            )
            nc.vector.tensor_tensor(OT[:, h, :], otr[0:E, :], pb[:], ALU.mult)

        # ---- phase 3: out proj + residual + global LN1 ----------------
        # Wo_s reuses a freed K^T slot (same pool tag, disjoint lifetime)
        Wo_s = kt_p.tile([64, H, D], bf16, tag="kt", name="Wo_s")
        nc.sync.dma_start(Wo_s[:], wos_d)

        def stats_collect(sums, src_qc, qc):
            """Per-block partial sum / sumsq, interleaved with the producer."""
            nc.vector.tensor_reduce(
                out=sums[:, qc:qc + 1], in_=src_qc, axis=AX.X, op=ALU.add
            )
            sqv = evac.tile([128, D], f32, tag="evac")
            nc.scalar.activation(
                sqv[:], src_qc, AF.Square, accum_out=sums[:, 4 + qc:5 + qc]
            )

        def stats_fire(sums, tag):
            """[sum, sumsq] -> AllGather (cheaper floor than AllReduce)."""
            pr = ps_po.tile([1, 8], f32, tag="po")
            nc.tensor.matmul(
                pr[:], lhsT=onesP[:], rhs=sums[:], start=True, stop=True
            )
            part = wk_p.tile([1, 2], f32, tag=f"part{tag}")
            nc.vector.tensor_reduce(
                out=part[:, 0:1], in_=pr[:, 0:4], axis=AX.X, op=ALU.add
            )
            nc.vector.tensor_reduce(
                out=part[:, 1:2], in_=pr[:, 4:8], axis=AX.X, op=ALU.add
            )
            cin = dram.tile([1, 2], f32)
            cout = dram.tile([N_CORES, 2], f32, addr_space="Shared")
            nc.sync.dma_start(cin[:], part[:])
            nc.gpsimd.collective_compute(
                "AllGather", ALU.bypass,
                replica_groups=[list(range(N_CORES))],
                ins=[cin[:]], outs=[cout[:]],
            )
            return cout

        z = xro  # in place: z = x + out
        sums1 = wk_p.tile([128, 8], f32, tag="sums_a")
        for qc in range(4):
            po = ps_mm.tile([128, D], f32, tag="mm")
            for h in range(H - 2):
                nc.tensor.matmul(
                    po[:],
                    lhsT=OT[:, h, qc * 128:(qc + 1) * 128],
                    rhs=Wo_s[:, h, :],
                    start=(h == 0), stop=False,
                )
            if qc == 0:
                # heads 6/7 finish normalizing while h0..5 streamed above
                for args in last_norm:
                    finish_norm(*args)
                last_norm = []
            for h in (H - 2, H - 1):
                nc.tensor.matmul(
                    po[:],
                    lhsT=OT[:, h, qc * 128:(qc + 1) * 128],
                    rhs=Wo_s[:, h, :],
                    start=False, stop=False,
                )
            nc.tensor.matmul(
                po[:], lhsT=ones_row[:], rhs=bo_r[:], start=False, stop=True
            )
            nc.vector.tensor_tensor(z[:, qc, :], po[:], xro[:, qc, :], ALU.add)
            stats_collect(sums1, z[:, qc, :], qc)

        def stats_finish(cout, tag):
            """-> [128, 2] sbuf tile: [:,0]=rstd, [:,1]=-mu*rstd (global)."""
            tot8 = wk_p.tile([N_CORES, 2], f32, tag=f"tot8{tag}")
            nc.sync.dma_start(tot8[:], cout[:])
            pr8 = ps_po.tile([1, 2], f32, tag="po")
            nc.tensor.matmul(
                pr8[:], lhsT=onesP[0:N_CORES, :], rhs=tot8[:],
                start=True, stop=True,
            )
            tot = wk_p.tile([1, 2], f32, tag=f"tot{tag}")
            nc.vector.tensor_copy(tot[:], pr8[:])
            sc = wk_p.tile([1, 6], f32, tag=f"sc{tag}")
            mu, m2 = sc[0:1, 0:1], sc[0:1, 1:2]
            nc.vector.tensor_scalar_mul(mu, tot[0:1, 0:1], INV_SD)
            nc.vector.tensor_scalar_mul(m2, tot[0:1, 1:2], INV_SD)
            nc.vector.tensor_tensor(sc[0:1, 2:3], mu, mu, ALU.mult)
            nc.vector.tensor_tensor(sc[0:1, 3:4], m2, sc[0:1, 2:3], ALU.subtract)
            nc.scalar.activation(sc[0:1, 4:5], sc[0:1, 3:4], AF.Sqrt, bias=eps_t[:])
            st2 = wk_p.tile([1, 2], f32r, tag=f"st2{tag}")
            nc.vector.reciprocal(st2[0:1, 0:1], sc[0:1, 4:5])        # rstd
            nc.vector.tensor_tensor(sc[0:1, 5:6], mu, st2[0:1, 0:1], ALU.mult)
            nc.vector.tensor_scalar_mul(st2[0:1, 1:2], sc[0:1, 5:6], -1.0)
            pbc = ps_po.tile([128, 2], f32, tag="po")
            nc.tensor.matmul(pbc[:], lhsT=ones_row_r[:], rhs=st2[:],
                             start=True, stop=True)
            stb = wk_p.tile([128, 2], f32, tag=f"stb{tag}")
            nc.vector.tensor_copy(stb[:], pbc[:])
            return stb

        cout1 = stats_fire(sums1, "a")
        # AR1 latency window: z^T transposes, then z^T @ W1 (the LN1 affine
        # commutes into the matmul in the fast_ln path)
        zT = single.tile([128, 4, R], bf16)        # z^T for the MLP path
        for dc in range(4):
            for qc in range(4):
                ptr = ps_po.tile([128, 128], f32, tag="po")
                nc.tensor.transpose(
                    ptr[:], z[:, qc, dc * 128:(dc + 1) * 128], ident[:]
                )
                nc.vector.tensor_copy(zT[:, dc, qc * 128:(qc + 1) * 128], ptr[:])
            # one quarter of the Kp projection per transpose group: keeps the
            # PE dense (the transpose+copy ladder alone idles it into a HAM
            # re-throttle)
            po = ps_mm.tile([128, D], f32, tag="mm")
            for mc in range(4):
                nc.tensor.matmul(
                    po[:],
                    lhsT=KTo[:, mc, dc * 128:(dc + 1) * 128],
                    rhs=Wo_p[:, mc, :],
                    start=(mc == 0), stop=False,
                )
            nc.tensor.matmul(
                po[:], lhsT=ones_row[:], rhs=bo_r[:], start=False, stop=True
            )
            ot = evac.tile([128, D], f32, tag="evac")
            nc.vector.tensor_copy(ot[:], po[:])
            nc.sync.dma_start(kp_v[:, dc, :], ot[:])

        h1T = single.tile([128, 16, R], bf16)
        if fast_ln:
            # zW1T = (z^T @ W1)^T in h1T's slot, relu-affine applied in place
            for fm in range(16):
                ph = ps_mm.tile([128, R], f32, tag="mm")
                for dc in range(4):
                    nc.tensor.matmul(
                        ph[:],
                        lhsT=W1_s[:, dc, fm * 128:(fm + 1) * 128],
                        rhs=zT[:, dc, :],
                        start=(dc == 0), stop=(dc == 3),
                    )
                nc.vector.tensor_copy(h1T[:, fm, :], ph[:])

        stb1 = stats_finish(cout1, "a")

        out1 = single.tile([128, 4, D], f32)
        if fast_ln:
            for qc in range(4):
                nc.scalar.activation(
                    out1[:, qc, :], z[:, qc, :], AF.Identity,
                    bias=stb1[:, 1:2], scale=stb1[:, 0:1],
                )
            # h1 = relu(rstd*zW1 + (b1 - mu*rstd*colsum(W1))) per f-partition
            bmlp = wk_p.tile([128, 16], f32, tag="bmlp")
            nc.vector.tensor_scalar(
                bmlp[:], cs1[:], stb1[0:128, 1:2], None, ALU.mult
            )
            nc.vector.tensor_tensor(bmlp[:], bmlp[:], b1s[:], ALU.add)
            for fm in range(16):
                nc.scalar.activation(
                    h1T[:, fm, :], h1T[:, fm, :], AF.Relu,
                    bias=bmlp[:, fm:fm + 1], scale=stb1[:, 0:1],
                )
        else:
            g_nat = single.tile([128, 4, D], bf16)
            nc.sync.dma_start(g_nat[:], gnat_d)
            b_nat = single.tile([128, 4, D], bf16)
            nc.sync.dma_start(b_nat[:], bnat_d)
            for qc in range(4):
                n_t = evac.tile([128, D], f32, tag="evac")
                nc.scalar.activation(
                    n_t[:], z[:, qc, :], AF.Identity,
                    bias=stb1[:, 1:2], scale=stb1[:, 0:1],
                )
                nc.vector.tensor_tensor(n_t[:], n_t[:], g_nat[:, qc, :], ALU.mult)
                nc.vector.tensor_tensor(
                    out1[:, qc, :], n_t[:], b_nat[:, qc, :], ALU.add)
            out1T = single.tile([128, 4, R], bf16)
            for dc in range(4):
                gT_t = evac.tile([128, R], bf16, tag="evacT")
                nc.sync.dma_start(gT_t[:], gT_d[:, dc, :])
                bT_t = evac.tile([128, R], bf16, tag="evacT")
                nc.sync.dma_start(bT_t[:], bT_d[:, dc, :])
                nT = evac.tile([128, R], bf16, tag="evacT")
                nc.scalar.activation(
                    nT[:], zT[:, dc, :], AF.Identity,
                    bias=stb1[:, 1:2], scale=stb1[:, 0:1],
                )
                nc.vector.tensor_tensor(nT[:], nT[:], gT_t[:], ALU.mult)
                nc.vector.tensor_tensor(out1T[:, dc, :], nT[:], bT_t[:], ALU.add)
            for fm in range(16):
                ph = ps_mm.tile([128, R], f32, tag="mm")
                for dc in range(4):
                    nc.tensor.matmul(
                        ph[:],
                        lhsT=W1_s[:, dc, fm * 128:(fm + 1) * 128],
                        rhs=out1T[:, dc, :],
                        start=(dc == 0), stop=(dc == 3),
                    )
                nc.scalar.activation(
                    h1T[:, fm, :], ph[:], AF.Relu, bias=b1s[:, fm:fm + 1]
                )

        # ---- phase 4: MLP second half + residual + global LN2 ---------
        w = out1  # in place: w = out1 + out2
        sums2 = wk_p.tile([128, 8], f32, tag="sums_b")
        for qc in range(4):
            po = ps_mm.tile([128, D], f32, tag="mm")
            for fm in range(16):
                nc.tensor.matmul(
                    po[:],
                    lhsT=h1T[:, fm, qc * 128:(qc + 1) * 128],
                    rhs=W2_s[:, fm // 4, fm % 4, :],
                    start=(fm == 0), stop=False,
                )
            nc.tensor.matmul(
                po[:], lhsT=ones_row[:], rhs=b2_r[:], start=False, stop=True
            )
            nc.vector.tensor_tensor(w[:, qc, :], po[:], out1[:, qc, :], ALU.add)
            stats_collect(sums2, w[:, qc, :], qc)

        cout2 = stats_fire(sums2, "b")
        # Vp: independent of both LayerNorms — fills the LN2-stats
        # collective latency window at the end of the kernel
        wo_project_packed(VTo, vp_v)
        stb2 = stats_finish(cout2, "b")
        for qc in range(4):
            n_t = evac.tile([128, D], f32, tag="evac")
            nc.scalar.activation(
                n_t[:], w[:, qc, :], AF.Identity,
                bias=stb2[:, 1:2], scale=stb2[:, 0:1],
            )
            if not fast_ln:
                nc.vector.tensor_tensor(n_t[:], n_t[:], g_nat[:, qc, :], ALU.mult)
                nc.vector.tensor_tensor(n_t[:], n_t[:], b_nat[:, qc, :], ALU.add)
            nc.sync.dma_start(fin_v[:, qc, :], n_t[:])

    split_waits(nc)
    return nc


def _prep(inp, fast_ln):
    """Host-side layout prep: cast weights to bf16 and pre-arrange into the
    exact SBUF layouts the kernel uses."""
    f = {k: np.ascontiguousarray(np.asarray(v, dtype=np.float32))
         for k, v in inp.items()}

    def tile128(a):  # [(c 128), n] -> [128, c, n]
        c = a.shape[0] // 128
        return np.ascontiguousarray(
            a.reshape(c, 128, a.shape[1]).transpose(1, 0, 2))

    def pack_heads(w):  # [H, D, E] -> [D, 512] with he = (h//2)*128+(h%2)*64+e
        out = np.zeros((D, D), np.float32)
        for h in range(H):
            out[:, (h // 2) * 128 + (h % 2) * 64:
                (h // 2) * 128 + (h % 2) * 64 + E] = w[h]
        return out

    def pack2(b):  # [H, E] -> [128, 4], p = (h%2)*64+e
        return np.ascontiguousarray(
            b.reshape(4, 2, E).transpose(1, 2, 0).reshape(128, 4))

    shared = dict(
        wq=tile128(pack_heads(f["Wq"])).astype(BF16),
        wk=tile128(pack_heads(f["Wk"])).astype(BF16),
        wv=tile128(pack_heads(f["Wv"])).astype(BF16),
        wo_s=np.ascontiguousarray(
            f["Wo"].reshape(H, E, D).transpose(1, 0, 2)).astype(BF16),
        wo_p=np.ascontiguousarray(
            f["Wo"].reshape(4, 2, E, D).transpose(1, 2, 0, 3)
            .reshape(128, 4, D)).astype(BF16),
        w1=tile128(f["W1"]).astype(BF16),
        w2=np.ascontiguousarray(
            f["W2"].reshape(4, 4, 128, D).transpose(2, 0, 1, 3)).astype(BF16),
        colsum_w1=np.ascontiguousarray(
            f["W1"].astype(BF16).astype(np.float32).sum(0)
            .reshape(16, 128).T),
        bqs2=pack2(f["bq"]),
        bks2=pack2(f["bk"]),
        bvs2=pack2(f["bv"]),
        bv_bc=np.ascontiguousarray(np.tile(f["bv"].reshape(1, D), (128, 1))),
        b1s=np.ascontiguousarray(f["b1"].reshape(16, 128).T),
        bo_r=f["bo"].reshape(1, D).astype(BF16),
        b2_r=f["b2"].reshape(1, D).astype(BF16),
    )

    def per_core(c):
        rows = slice(c * R, (c + 1) * R)
        xr = f["x"][rows]
        m = dict(
            xro=tile128(xr),
            xrT=tile128(np.ascontiguousarray(xr.T)).astype(BF16),
            **shared,
        )
        if not fast_ln:
            m.update(
                g_nat=tile128(f["ln_g"][rows]).astype(BF16),
                b_nat=tile128(f["ln_b"][rows]).astype(BF16),
                gT=tile128(np.ascontiguousarray(f["ln_g"][rows].T)).astype(BF16),
                bT=tile128(np.ascontiguousarray(f["ln_b"][rows].T)).astype(BF16),
            )
        return m

    return [per_core(c) for c in range(N_CORES)]


_NC_CACHE = {}


def _fast_ln_ok(inputs):
    return bool(
        np.all(np.asarray(inputs["ln_g"]) == 1.0)
        and np.all(np.asarray(inputs["ln_b"]) == 0.0)
    )


def _get_nc(fast_ln=True):
    if fast_ln not in _NC_CACHE:
        _NC_CACHE[fast_ln] = build_nc(fast_ln)
    return _NC_CACHE[fast_ln]


def make_in_maps(inputs):
    return _prep(inputs, _fast_ln_ok(inputs))


def kernel(**inputs):
    fast_ln = _fast_ln_ok(inputs)
    in_maps = _prep(inputs, fast_ln)
    nc = _get_nc(fast_ln)
    res = run_bass_kernel_spmd(nc, in_maps, list(range(N_CORES)))
    final = np.concatenate([res.results[c]["final_rows"] for c in range(N_CORES)])
    Kp = np.concatenate([res.results[c]["Kp_rows"] for c in range(N_CORES)])
    Vp = np.concatenate([res.results[c]["Vp_rows"] for c in range(N_CORES)])
    return (final, Kp, Vp)
